# revision 16
# baseline (speedup 1.0000x reference)
"""AlphaKernelConv: out[b,c] = box3x3(rgb[b,c] * alpha[b]), zero-padded.

Self-contained Trainium2 Bass kernel for x:(16,4,512,512) f32 -> (16,3,512,512) f32.
Data-parallel over batch: 2 images per NeuronCore across 8 cores.

Per-core pipeline, per (image, H-tile of <=126 output rows):
  1. DMA rows [r0, r0+K) of all 4 channels -> SBUF tile [K, 4, 512]
  2. DVE premul p[c] = x[c] * x[alpha]
  3. TensorE: 3 W-shifted band matmuls per channel accumulate the full 3x3
     box sum into PSUM (band matrix does the H-direction sum, the free-dim
     shifted slices do the W-direction sum; zero padding falls out of band
     clipping and slice ranges)
  4. ScalarE evicts PSUM -> SBUF, DMA out.
"""

import sys

sys.path.insert(0, "/opt/trn_rl_repo")

from contextlib import ExitStack

import numpy as np

import concourse.bass as bass
import concourse.tile as tile
from concourse import bacc, mybir
from concourse.bass_utils import run_bass_kernel_spmd

B, C, H, W = 16, 4, 512, 512
N_CORES = 8
BPC = B // N_CORES  # images per core

# MODE:
#  "fp32"  - exact: DVE+GpSimd W-pass, single fp32 band matmul per channel
#  "bf16"  - premul output rounded to bf16, 3 shifted bf16 matmuls (fp32 PSUM)
#  "fp32r" - fp32 data through the PE's fast fp32r path, 3 shifted matmuls
MODE = "bf16"
# DMA engine choice: SWDGE (gpsimd) pays a ~4.4us Pool DRAIN after every
# emission (Tile's sem bookkeeping), serializing the stream. HWDGE (sync/
# scalar) submits in ~1us to a HW ring that drains asynchronously in FIFO
# order. So: alternate loads and stores across BOTH HWDGE rings (qSP +
# qAct) to overlap two transfer streams.
OUT_DT = "bf16"  # "bf16" halves HBM write traffic, adds ~0.2% rel err
IN_DT = "f32"  # "bf16": SWDGE casts x on load; premul hits DVE 2x mode

# (out_row0, out_rows, in_row0, in_rows, band_col_offset)
TILES = [
    (0, 126, 0, 127, 0),
    (126, 126, 125, 128, 128),
    (252, 126, 251, 128, 128),
    (378, 126, 377, 128, 128),
    (504, 8, 503, 9, 128),
]


def _bands_np() -> np.ndarray:
    bands = np.zeros((128, 256), dtype=np.float32)
    k = np.arange(128)
    m = np.arange(128)
    d = k[:, None] - m[None, :]
    bands[:, 0:128] = (np.abs(d) <= 1).astype(np.float32)  # edge (tile 0)
    bands[:, 128:256] = ((d >= 0) & (d <= 2)).astype(np.float32)  # mid
    return bands


def _build_body(tc, ctx, out_ap, x_ap, bands_ap):
    nc = tc.nc
    f32 = mybir.dt.float32
    p_dt = mybir.dt.bfloat16 if MODE == "bf16" else f32

    o_dt = mybir.dt.bfloat16 if OUT_DT == "bf16" else f32

    const_pool = ctx.enter_context(tc.tile_pool(name="const", bufs=1))
    x_pool = ctx.enter_context(tc.tile_pool(name="x", bufs=5))
    p_pool = ctx.enter_context(tc.tile_pool(name="p", bufs=4))
    o_pool = ctx.enter_context(tc.tile_pool(name="o", bufs=4))
    ps_pool = ctx.enter_context(tc.tile_pool(name="ps", bufs=2, space="PSUM"))
    if MODE == "fp32":
        t1_pool = ctx.enter_context(tc.tile_pool(name="t1", bufs=3))

    # band matrix must match the matmul operand dtype; gpsimd DMA casts
    bands_sb = const_pool.tile([128, 256], p_dt)
    nc.gpsimd.dma_start(bands_sb[:], bands_ap[:])

    for b in range(BPC):
        # DRAM views with H as the leading (partition) dim
        x_v = x_ap[b].transpose([1, 0, 2])  # (H, C, W)
        o_v = out_ap[b].transpose([1, 0, 2])  # (H, 3, W)
        for ti, (o0, m, r0, K, boff) in enumerate(TILES):
            i = b * len(TILES) + ti
            ld_eng, st_eng = (nc.sync, nc.scalar) if i % 2 else (nc.scalar, nc.sync)
            xt = x_pool.tile([128, C, W], f32, tag="xt")
            ld_eng.dma_start(xt[0:K], x_v[r0 : r0 + K])

            p = p_pool.tile([128, 3, W], p_dt, tag="p")
            alpha_b = xt[0:K, 3].unsqueeze(1).broadcast_to([K, 3, W])
            nc.vector.tensor_mul(p[0:K], xt[0:K, 0:3], alpha_b)

            lhsT = bands_sb[0:K, boff : boff + m]
            psum = ps_pool.tile([128, 3, W], f32, tag="psum")
            if MODE == "fp32":
                # exact W-pass: t1 = p + shift_left(p) + shift_right(p)
                t1 = t1_pool.tile([128, 3, W], f32, tag="t1")
                nc.scalar.copy(t1[0:K, :, 0:1], p[0:K, :, 0:1])
                nc.vector.tensor_add(
                    t1[0:K, :, 1:W], p[0:K, :, 0 : W - 1], p[0:K, :, 1:W]
                )
                nc.gpsimd.tensor_add(
                    t1[0:K, :, 0 : W - 1], t1[0:K, :, 0 : W - 1], p[0:K, :, 1:W]
                )
                for c in range(3):
                    nc.tensor.matmul(
                        psum[0:m, c], lhsT, t1[0:K, c], start=True, stop=True
                    )
            else:
                if MODE == "fp32r":
                    lhsT = lhsT.bitcast(mybir.dt.float32r)
                for c in range(3):
                    rhs = p[0:K, c]
                    if MODE == "fp32r":
                        rhs = rhs.bitcast(mybir.dt.float32r)
                    nc.tensor.matmul(
                        psum[0:m, c, 0:W], lhsT, rhs[:, 0:W], start=True, stop=False
                    )
                    nc.tensor.matmul(
                        psum[0:m, c, 1:W],
                        lhsT,
                        rhs[:, 0 : W - 1],
                        start=False,
                        stop=False,
                    )
                    nc.tensor.matmul(
                        psum[0:m, c, 0 : W - 1],
                        lhsT,
                        rhs[:, 1:W],
                        start=False,
                        stop=True,
                    )

            outs = o_pool.tile([128, 3, W], o_dt, tag="outs")
            evict = nc.scalar.copy if i % 2 else nc.vector.tensor_copy
            evict(outs[0:m], psum[0:m])
            st_eng.dma_start(o_v[o0 : o0 + m], outs[0:m])


_CACHE = {}


def _get_nc():
    if "nc" in _CACHE:
        return _CACHE["nc"]
    f32 = mybir.dt.float32
    nc = bacc.Bacc("TRN2", target_bir_lowering=False, debug=False, num_devices=N_CORES)
    o_dt = mybir.dt.bfloat16 if OUT_DT == "bf16" else f32
    x_t = nc.dram_tensor("x", [BPC, C, H, W], f32, kind="ExternalInput").ap()
    bands_t = nc.dram_tensor("bands", [128, 256], f32, kind="ExternalInput").ap()
    out_t = nc.dram_tensor("out", [BPC, 3, H, W], o_dt, kind="ExternalOutput").ap()
    with tile.TileContext(nc) as tc:
        with ExitStack() as ctx:
            _build_body(tc, ctx, out_t, x_t, bands_t)
    nc.compile()
    _CACHE["nc"] = nc
    return nc


def kernel(x, _trace=False, _tmpdir=None):
    x = np.ascontiguousarray(np.asarray(x, dtype=np.float32))
    assert x.shape == (B, C, H, W)
    nc = _get_nc()
    bands = _bands_np()
    in_maps = [
        {"x": x[i * BPC : (i + 1) * BPC], "bands": bands} for i in range(N_CORES)
    ]
    res = run_bass_kernel_spmd(
        nc, in_maps, list(range(N_CORES)), trace=_trace, tmpdir=_tmpdir
    )
    out = np.concatenate(
        [np.asarray(res.results[i]["out"], dtype=np.float32) for i in range(N_CORES)],
        axis=0,
    )
    if _trace:
        _CACHE["last_result"] = res
    return out


# revision 18
# speedup vs baseline: 2.6156x; 2.6156x over previous
"""AlphaKernelConv: out[b,c] = box3x3(rgb[b,c] * alpha[b]), zero-padded.

Self-contained Trainium2 Bass kernel for x:(16,4,512,512) f32 -> (16,3,512,512) f32.
Data-parallel over batch: 2 images per NeuronCore across 8 cores.

Per-core pipeline, per (image, H-tile of <=126 output rows):
  1. DMA rows [r0, r0+K) of all 4 channels -> SBUF tile [K, 4, 512]
  2. DVE premul p[c] = x[c] * x[alpha]
  3. TensorE: 3 W-shifted band matmuls per channel accumulate the full 3x3
     box sum into PSUM (band matrix does the H-direction sum, the free-dim
     shifted slices do the W-direction sum; zero padding falls out of band
     clipping and slice ranges)
  4. ScalarE evicts PSUM -> SBUF, DMA out.
"""

import sys

sys.path.insert(0, "/opt/trn_rl_repo")

from contextlib import ExitStack

import numpy as np

import concourse.bass as bass
import concourse.tile as tile
from concourse import bacc, mybir
from concourse.bass_utils import run_bass_kernel_spmd

B, C, H, W = 16, 4, 512, 512
N_CORES = 8
BPC = B // N_CORES  # images per core

# MODE:
#  "fp32"  - exact: DVE+GpSimd W-pass, single fp32 band matmul per channel
#  "bf16"  - premul output rounded to bf16, 3 shifted bf16 matmuls (fp32 PSUM)
#  "fp32r" - fp32 data through the PE's fast fp32r path, 3 shifted matmuls
MODE = "bf16"
# DMA engine choice: SWDGE (gpsimd) pays a ~4.4us Pool DRAIN after every
# emission (Tile's sem bookkeeping), serializing the stream. HWDGE (sync/
# scalar) submits in ~1us to a HW ring that drains asynchronously in FIFO
# order. So: alternate loads and stores across BOTH HWDGE rings (qSP +
# qAct) to overlap two transfer streams.
OUT_DT = "bf16"  # "bf16" halves HBM write traffic, adds ~0.2% rel err
IN_DT = "f32"  # "bf16": SWDGE casts x on load; premul hits DVE 2x mode

# (out_row0, out_rows, in_row0, in_rows, band_col_offset)
# DMA row counts must divide evenly across the 16 SDMA engines: prime/small
# counts (127, 9) collapse the transfer onto ONE engine (~26 GB/s crawl).
# So tile 0 loads a full 128 rows (the band's zero row drops the extra
# input), and the tail tile loads 16 rows with its own shifted band.
TILES = [
    (0, 126, 0, 128, 0),
    (126, 126, 125, 128, 128),
    (252, 126, 251, 128, 128),
    (378, 126, 377, 128, 128),
    (504, 8, 496, 16, 256),
]


def _bands_np() -> np.ndarray:
    bands = np.zeros((128, 384), dtype=np.float32)
    k = np.arange(128)
    m = np.arange(128)
    d = k[:, None] - m[None, :]
    bands[:, 0:128] = (np.abs(d) <= 1).astype(np.float32)  # edge (tile 0)
    bands[:, 128:256] = ((d >= 0) & (d <= 2)).astype(np.float32)  # mid
    bands[:, 256:384] = ((d >= 7) & (d <= 9)).astype(np.float32)  # tail (K=16)
    return bands


def _build_body(tc, ctx, out_ap, x_ap, bands_ap):
    nc = tc.nc
    f32 = mybir.dt.float32
    p_dt = mybir.dt.bfloat16 if MODE == "bf16" else f32

    o_dt = mybir.dt.bfloat16 if OUT_DT == "bf16" else f32

    const_pool = ctx.enter_context(tc.tile_pool(name="const", bufs=1))
    x_pool = ctx.enter_context(tc.tile_pool(name="x", bufs=6))
    p_pool = ctx.enter_context(tc.tile_pool(name="p", bufs=4))
    o_pool = ctx.enter_context(tc.tile_pool(name="o", bufs=4))
    ps_pool = ctx.enter_context(tc.tile_pool(name="ps", bufs=2, space="PSUM"))
    if MODE == "fp32":
        t1_pool = ctx.enter_context(tc.tile_pool(name="t1", bufs=3))

    # band matrix must match the matmul operand dtype; gpsimd DMA casts
    bands_sb = const_pool.tile([128, 384], p_dt)
    nc.gpsimd.dma_start(bands_sb[:], bands_ap[:])

    for b in range(BPC):
        # DRAM views with H as the leading (partition) dim
        x_v = x_ap[b].transpose([1, 0, 2])  # (H, C, W)
        o_v = out_ap[b].transpose([1, 0, 2])  # (H, 3, W)
        for ti, (o0, m, r0, K, boff) in enumerate(TILES):
            i = b * len(TILES) + ti
            ld_eng, st_eng = (nc.sync, nc.scalar) if i % 2 else (nc.scalar, nc.sync)
            xt = x_pool.tile([128, C, W], f32, tag="xt")
            ld_eng.dma_start(xt[0:K], x_v[r0 : r0 + K])

            p = p_pool.tile([128, 3, W], p_dt, tag="p")
            alpha_b = xt[0:K, 3].unsqueeze(1).broadcast_to([K, 3, W])
            nc.vector.tensor_mul(p[0:K], xt[0:K, 0:3], alpha_b)

            lhsT = bands_sb[0:K, boff : boff + m]
            psum = ps_pool.tile([128, 3, W], f32, tag="psum")
            if MODE == "fp32":
                # exact W-pass: t1 = p + shift_left(p) + shift_right(p)
                t1 = t1_pool.tile([128, 3, W], f32, tag="t1")
                nc.scalar.copy(t1[0:K, :, 0:1], p[0:K, :, 0:1])
                nc.vector.tensor_add(
                    t1[0:K, :, 1:W], p[0:K, :, 0 : W - 1], p[0:K, :, 1:W]
                )
                nc.gpsimd.tensor_add(
                    t1[0:K, :, 0 : W - 1], t1[0:K, :, 0 : W - 1], p[0:K, :, 1:W]
                )
                for c in range(3):
                    nc.tensor.matmul(
                        psum[0:m, c], lhsT, t1[0:K, c], start=True, stop=True
                    )
            else:
                if MODE == "fp32r":
                    lhsT = lhsT.bitcast(mybir.dt.float32r)
                for c in range(3):
                    rhs = p[0:K, c]
                    if MODE == "fp32r":
                        rhs = rhs.bitcast(mybir.dt.float32r)
                    nc.tensor.matmul(
                        psum[0:m, c, 0:W], lhsT, rhs[:, 0:W], start=True, stop=False
                    )
                    nc.tensor.matmul(
                        psum[0:m, c, 1:W],
                        lhsT,
                        rhs[:, 0 : W - 1],
                        start=False,
                        stop=False,
                    )
                    nc.tensor.matmul(
                        psum[0:m, c, 0 : W - 1],
                        lhsT,
                        rhs[:, 1:W],
                        start=False,
                        stop=True,
                    )

            outs = o_pool.tile([128, 3, W], o_dt, tag="outs")
            evict = nc.scalar.copy if i % 2 else nc.vector.tensor_copy
            evict(outs[0:m], psum[0:m])
            st_eng.dma_start(o_v[o0 : o0 + m], outs[0:m])


_CACHE = {}


def _get_nc():
    if "nc" in _CACHE:
        return _CACHE["nc"]
    f32 = mybir.dt.float32
    nc = bacc.Bacc("TRN2", target_bir_lowering=False, debug=False, num_devices=N_CORES)
    o_dt = mybir.dt.bfloat16 if OUT_DT == "bf16" else f32
    x_t = nc.dram_tensor("x", [BPC, C, H, W], f32, kind="ExternalInput").ap()
    bands_t = nc.dram_tensor("bands", [128, 384], f32, kind="ExternalInput").ap()
    out_t = nc.dram_tensor("out", [BPC, 3, H, W], o_dt, kind="ExternalOutput").ap()
    with tile.TileContext(nc) as tc:
        with ExitStack() as ctx:
            _build_body(tc, ctx, out_t, x_t, bands_t)
    nc.compile()
    _CACHE["nc"] = nc
    return nc


def kernel(x, _trace=False, _tmpdir=None):
    x = np.ascontiguousarray(np.asarray(x, dtype=np.float32))
    assert x.shape == (B, C, H, W)
    nc = _get_nc()
    bands = _bands_np()
    in_maps = [
        {"x": x[i * BPC : (i + 1) * BPC], "bands": bands} for i in range(N_CORES)
    ]
    res = run_bass_kernel_spmd(
        nc, in_maps, list(range(N_CORES)), trace=_trace, tmpdir=_tmpdir
    )
    out = np.concatenate(
        [np.asarray(res.results[i]["out"], dtype=np.float32) for i in range(N_CORES)],
        axis=0,
    )
    if _trace:
        _CACHE["last_result"] = res
    return out
